# revision 20
# baseline (speedup 1.0000x reference)
"""Trainium2 Bass kernel for CRF negative log-likelihood (nn_CRF).

Strategy:
  - data-parallel over batch: 8 cores x 16 sequences each.
  - forward algorithm in the exp domain, fwd and bwd chains STACKED on the
    partition axis (fwd state at partitions 0:52, bwd at 64:116) so each of
    the 128 serial steps is ONE bf16 matmul against a constant block-diagonal
    (128,128) stationary + ONE vector multiply by the per-step emission slab.
    bf16 halves PE work (fp32 matmuls are 2 HW passes) and the merged chain
    halves the cross-engine ping-pong count vs separate fwd/bwd chains.
  - one renorm at k=64, applied 3 steps later (k=67) so the colsum /
    reciprocal / broadcast sub-chain runs OFF the critical path; only one
    extra vector multiply lands on the chain.
  - masking via the absorbing-STOP construction (lengths >= S/2 so only the
    bwd half and t=128 need mask gating); per-step rescale exp(-C0) folded
    into the emission exps; host adds C0*len back.
  - prologue kept off the chain: constants (block-diagonal exp(trans),
    gold one-hots) are host-marshalled bf16, DMA issue is spread across the
    engine queues, and the emission ACTs are chunked so the scan starts as
    soon as the first slots land.
  - gold path score: one-hot pair counts via 32 PSUM-accumulated bf16
    matmuls that Tile schedules into the PE queue's idle slots (the w_last
    column is folded in as an extra one-hot column so end transitions need
    no separate matmuls); gold elementwise work runs on the otherwise-idle
    GPSIMD engine.
  - device emits per-core partials (renorm scales, midpoint dot, gold sums);
    the host does the final log/sum ("all-reduce").
"""

import numpy as np

TAG = 52
START, STOP = TAG - 2, TAG - 1
B, S = 128, 256
NCORES = 8
BL = B // NCORES            # 16 sequences per core
NK = 128                    # stacked chain length (fwd t=0..128, bwd t=255..129)
RK = 64                     # renorm colsum at step 64 ...
RKF = 67                    # ... applied (fixup multiply) at step 67
GK0 = 40                    # gold pair-matmul emission rides with scan steps
C0 = 4.9                    # constant per-step rescale (nats)
MGATE = 64.0                # mask gate constant (exp(-64) ~ 0)
M32 = (S * BL) // 128       # 32 free columns for (128, M32) gold layout
BW = 64                     # partition offset of the bwd block
FC = 32                     # first-chunk slot count for the emission builds

_CACHE: dict = {}


def _build_nc(debug: bool = False):
    import concourse.bass as bass
    import concourse.mybir as mybir
    import concourse.tile as tile
    from concourse import bacc

    f32 = mybir.dt.float32
    bf16 = mybir.dt.bfloat16
    nc = bacc.Bacc("TRN2", target_bir_lowering=False, debug=debug)

    # ---- external inputs (per-core shards, host-marshalled layouts) ----
    featsF = nc.dram_tensor("featsF", (BW, NK, BL), f32, kind="ExternalInput")
    featsB = nc.dram_tensor("featsB", (BW, NK, BL), f32, kind="ExternalInput")
    featsL = nc.dram_tensor("featsL", (TAG, BL), f32, kind="ExternalInput")
    colc = nc.dram_tensor("colc", (128, 3), f32, kind="ExternalInput")
    bdh = nc.dram_tensor("bdh", (128, 128), bf16, kind="ExternalInput")
    s2bh = nc.dram_tensor("s2bh", (2, 128), f32, kind="ExternalInput")
    yh = nc.dram_tensor("yh", (128, M32, TAG), bf16, kind="ExternalInput")
    ypah = nc.dram_tensor("ypah", (128, M32, TAG + 1), bf16, kind="ExternalInput")
    fnh = nc.dram_tensor("fnh", (128, M32, TAG), bf16, kind="ExternalInput")
    trph = nc.dram_tensor("trph", (TAG + 1, TAG), f32, kind="ExternalInput")

    # ---- external outputs ----
    # out_scan: row 0 = [renorm fwd scales | Ssum], row 1 = [renorm bwd | 0]
    out_scan = nc.dram_tensor("out_scan", (2, 2 * BL), f32, kind="ExternalOutput")
    # out_gold: [0,0] = emit partial, [0,1] = trans+end partial
    out_gold = nc.dram_tensor("out_gold", (1, 4), f32, kind="ExternalOutput")

    AL = mybir.AluOpType
    EXP = mybir.ActivationFunctionType.Exp

    with tile.TileContext(nc) as tc:
        with (
            tc.tile_pool(name="persist", bufs=1) as persist,
            tc.tile_pool(name="ustate", bufs=3) as ustate,
            tc.tile_pool(name="gold", bufs=1) as goldp,
            tc.tile_pool(name="psZ", bufs=2, space="PSUM") as psZ,
            tc.tile_pool(name="psG", bufs=1, space="PSUM") as psG,
            tc.tile_pool(name="psR", bufs=1, space="PSUM") as psR,
        ):
            # ---------- sync queue: chunked feature DMAs ----------
            # fStack: fwd logits at partitions 0:52, bwd (reversed t, mask
            # gate folded in host-side) at 64:116; chunks aligned with the
            # emission ACT chunks so the scan starts as soon as possible
            fStack = persist.tile([128, NK, BL], f32, name="fStack", tag="fStack")
            nc.sync.dma_start(out=fStack[0:BW, 0:FC, :], in_=featsF[:, 0:FC, :])
            nc.sync.dma_start(out=fStack[BW:128, 0:FC, :], in_=featsB[:, 0:FC, :])
            nc.sync.dma_start(out=fStack[0:BW, FC:NK, :], in_=featsF[:, FC:NK, :])
            nc.sync.dma_start(out=fStack[BW:128, FC:NK, :], in_=featsB[:, FC:NK, :])
            # gold one-hots after the scan-gating chunks (the pair matmuls
            # are dep-pinned behind scan step 40+, so these can land late)
            Ysb = goldp.tile([128, M32, TAG], bf16, name="Ysb", tag="Ysb")
            nc.sync.dma_start(out=Ysb, in_=yh[:, :, :])
            YPsb = goldp.tile([128, M32, TAG + 1], bf16, name="YPsb", tag="YPsb")
            nc.sync.dma_start(out=YPsb, in_=ypah[:, :, :])

            # ---------- scalar queue: colc/BDT DMAs then the emission ACTs ---
            colc_sb = persist.tile([128, 3], f32, name="colc_sb", tag="colc_sb")
            nc.scalar.dma_start(out=colc_sb, in_=colc[:, :])
            BDT = persist.tile([128, 128], bf16, name="BDT", tag="BDT")
            nc.scalar.dma_start(out=BDT, in_=bdh[:, :])
            biasgF = colc_sb[0:TAG, 1:2]           # DlastF bias
            biasU0 = colc_sb[0:TAG, 2:3]           # trans[START,:] - C0

            # ---------- vector queue: small consts only ----------
            s2b = persist.tile([2, 128], f32, name="s2b", tag="s2b")
            nc.gpsimd.dma_start(out=s2b, in_=s2bh[:, :])
            S2a = persist.tile([128, 2], bf16, name="S2a", tag="S2a")
            nc.vector.memset(S2a, 0.0)
            nc.vector.memset(S2a[0:TAG, 0:1], 1.0)
            nc.vector.memset(S2a[BW : BW + TAG, 1:2], 1.0)
            onesc = persist.tile([TAG, 1], bf16, name="onesc", tag="onesc")
            nc.vector.memset(onesc, 1.0)
            stage2 = persist.tile([2, 2 * BL], f32, name="stage2", tag="stage2")

            # ---------- gpsimd queue: U0 one-hot, Dst dead rows, late DMAs ---
            U0 = persist.tile([128, BL], bf16, name="U0", tag="U0")
            onebl = persist.tile([128, BL], bf16, name="onebl", tag="onebl")
            nc.gpsimd.memset(onebl, 1.0)
            nc.gpsimd.affine_select(
                out=U0,
                in_=onebl,
                compare_op=AL.is_equal,
                fill=0.0,
                base=-(BW + STOP),
                pattern=[[0, BL]],
                channel_multiplier=1,
            )
            # dead rows 52:64 / 116:128 get exp(0+0)=1 from the 64-row ACTs
            # (zeros marshalled into featsF/featsB rows 52:64); finite is all
            # the chain needs there -- BDT's zero blocks null them out
            Dst = persist.tile([128, NK, BL], f32, name="Dst", tag="Dst")
            fL = persist.tile([TAG, BL], f32, name="fL", tag="fL")
            nc.gpsimd.dma_start(out=fL, in_=featsL[:, :])
            FNsb = goldp.tile([128, M32, TAG], bf16, name="FNsb", tag="FNsb")
            nc.gpsimd.dma_start(out=FNsb, in_=fnh[:, :, :])
            trP = goldp.tile([TAG + 1, TAG], f32, name="trP", tag="trP")
            nc.gpsimd.dma_start(out=trP, in_=trph[:, :])

            # ---------- emission exps (mask gates pre-applied host-side) -----
            # U0 fwd rows = X_0 = exp(f_0 + trans[START,:] - C0) (ACT after the
            # gpsimd one-hot write; Tile orders the overlapping writes)
            nc.scalar.activation(
                out=U0[0:TAG, :], in_=fStack[0:TAG, 0, :], func=EXP, bias=biasU0
            )
            nc.scalar.activation(
                out=Dst[0:BW, 0:FC, :],
                in_=fStack[0:BW, 0:FC, :],
                func=EXP,
                bias=colc_sb[0:BW, 0:1],
            )
            nc.scalar.activation(
                out=Dst[BW:128, 0:FC, :],
                in_=fStack[BW:128, 0:FC, :],
                func=EXP,
                bias=colc_sb[BW:128, 0:1],
            )
            nc.scalar.activation(
                out=Dst[0:BW, FC:NK, :],
                in_=fStack[0:BW, FC:NK, :],
                func=EXP,
                bias=colc_sb[0:BW, 0:1],
            )
            nc.scalar.activation(
                out=Dst[BW:128, FC:NK, :],
                in_=fStack[BW:128, FC:NK, :],
                func=EXP,
                bias=colc_sb[BW:128, 0:1],
            )
            # DlastF: gated d(t=128) at partitions 0:52
            DlastF = persist.tile([TAG, BL], f32, name="DlastF", tag="DlastF")
            nc.scalar.activation(out=DlastF, in_=fL, func=EXP, bias=biasgF)

            # ---------- gold emit partial (gpsimd, fully off-path) ----------
            stage_gold = goldp.tile([1, 4], f32, name="stage_gold", tag="stage_gold")
            nc.gpsimd.memset(stage_gold, 0.0)
            scrap = goldp.tile([128, M32, TAG], f32, name="scrap", tag="scrap")
            nc.gpsimd.tensor_mul(scrap, Ysb, FNsb)
            nc.gpsimd.tensor_reduce(
                out=stage_gold[0:1, 0:1],
                in_=scrap,
                axis=mybir.AxisListType.XYZWC,
                op=AL.add,
            )

            # ================= the stacked scan ==============================
            cnt_a = psG.tile([TAG + 1, TAG], f32, name="cnt_a", tag="cnt_a")
            ps_s2 = psR.tile([2, BL], f32, name="ps_s2", tag="ps_s2")
            ps_bc = psR.tile([128, BL], f32, name="ps_bc", tag="ps_bc")
            rcp2 = persist.tile([2, BL], f32, name="rcp2", tag="rcp2")
            cnt_sb = goldp.tile([TAG + 1, TAG], f32, name="cnt_sb", tag="cnt_sb")

            Zp = psZ.tile([128, BL], f32, name="Z0", tag="Z")
            nc.tensor.matmul(Zp, BDT, U0, start=True, stop=True)

            Ulast = None
            for k in range(1, NK):
                U = ustate.tile([128, BL], bf16, name=f"U{k}", tag="U")
                nc.vector.tensor_tensor(out=U, in0=Dst[:, k, :], in1=Zp, op=AL.mult)
                if k == RKF:
                    # apply the k=64 renorm scales (off-critical-path sub-chain)
                    Uf = ustate.tile([128, BL], bf16, name=f"Uf{k}", tag="U")
                    nc.vector.tensor_tensor(out=Uf, in0=U, in1=ps_bc, op=AL.mult)
                    U = Uf
                Z = psZ.tile([128, BL], f32, name=f"Z{k}", tag="Z")
                zinst = nc.tensor.matmul(Z, BDT, U, start=True, stop=True)
                if k == RK:
                    # colsum of U_64 per half -> scales; reciprocal; broadcast
                    nc.tensor.matmul(ps_s2, S2a, U, start=True, stop=True)
                if k == RK + 2:
                    nc.vector.reciprocal(rcp2, ps_s2)
                    nc.tensor.matmul(ps_bc, s2b, rcp2, start=True, stop=True)
                if k == RK + 6:
                    nc.vector.tensor_copy(stage2[0:2, 0:BL], ps_s2)
                if k == NK - 1:
                    Ulast = U
                if GK0 <= k < GK0 + 2 * M32 and (k - GK0) % 2 == 0:
                    m = (k - GK0) // 2
                    pinst = nc.tensor.matmul(
                        cnt_a,
                        YPsb[:, m, :],
                        Ysb[:, m, :],
                        start=(m == 0),
                        stop=(m == M32 - 1),
                        skip_group_check=True,
                    )
                    # same-engine ordering edge: without it the static
                    # scheduler hoists the whole pair chain to the front of
                    # the PE queue, where its DMA wait stalls the scan
                    tile.add_dep_helper(
                        pinst.ins, zinst.ins, sync=False,
                        reason="pin gold pair MM behind its scan step",
                    )
                if k == GK0 + 2 * M32 + 1:
                    # bounce the finished pair counts to SBUF for gpsimd
                    nc.vector.tensor_copy(cnt_sb, cnt_a)
                Zp = Z

            # ================= tail ==========================================
            # land the bwd half of Z_127 at partitions 0:52 via PE (column
            # slice of the stationary selects the bwd block)
            ps_zb = psR.tile([TAG, BL], f32, name="ps_zb", tag="ps_zb")
            nc.tensor.matmul(ps_zb, BDT[:, BW : BW + TAG], Ulast, start=True, stop=True)
            U128f = persist.tile([TAG, BL], bf16, name="U128f", tag="U128f")
            nc.vector.tensor_tensor(
                out=U128f, in0=DlastF, in1=Zp[0:TAG, :], op=AL.mult
            )
            Pm = persist.tile([TAG, BL], bf16, name="Pm", tag="Pm")
            nc.vector.tensor_tensor(out=Pm, in0=U128f, in1=ps_zb, op=AL.mult)
            ps_sm = psR.tile([1, BL], f32, name="ps_sm", tag="ps_sm")
            nc.tensor.matmul(ps_sm, onesc, Pm, start=True, stop=True)
            # (DMA cannot read PSUM: bounce through SBUF; unused out_scan
            # region is never read by the host)
            nc.vector.tensor_copy(stage2[0:1, BL : 2 * BL], ps_sm)
            nc.sync.dma_start(out=out_scan[:, :], in_=stage2)

            # gold tail: (pair+end counts) . weights — runs mid-scan
            scrapA = goldp.tile([TAG + 1, TAG], f32, name="scrapA", tag="scrapA")
            nc.gpsimd.tensor_mul(scrapA, cnt_sb, trP)
            nc.gpsimd.tensor_reduce(
                out=stage_gold[0:1, 1:2],
                in_=scrapA,
                axis=mybir.AxisListType.XYZWC,
                op=AL.add,
            )
            nc.sync.dma_start(out=out_gold[:, :], in_=stage_gold)

    nc.compile()
    return nc


def _prep_core_inputs(feats, transitions, mask, tags, core):
    """Host marshalling of the core's batch shard (layouts, dtype casts,
    and the tiny constant tables derived from the 52x52 transitions)."""
    import ml_dtypes

    f32 = np.float32
    bf = ml_dtypes.bfloat16
    sl = slice(core * BL, (core + 1) * BL)
    f = np.ascontiguousarray(feats[sl]).astype(f32, copy=False)   # (BL,S,T)
    m = mask[sl].astype(f32)                                      # (BL,S)
    tg = tags[sl].astype(f32)                                     # (BL,S)

    fT = np.ascontiguousarray(f.transpose(2, 1, 0)).copy()        # (T,S,BL)
    fT[STOP] = 0.0
    idxB = np.array([128] + list(range(S - 1, 128, -1)))          # t=128,255..129
    # fwd/bwd logit blocks padded to 64 partitions (rows 52:64 zero) so the
    # exp ACTs can cover the stacked layout's dead rows; mask gate folded
    # into the bwd block: logits + MGATE*m (-MGATE on the STOP row)
    featsF = np.zeros((BW, NK, BL), f32)
    featsF[0:TAG] = fT[:, 0:NK, :]                                # slots t=0..127
    sg = np.full((TAG, 1, 1), MGATE, f32)
    sg[STOP] = -MGATE
    featsB = np.zeros((BW, NK, BL), f32)
    featsB[0:TAG] = fT[:, idxB, :] + sg * m.T[idxB][None, :, :]
    featsL = np.ascontiguousarray(featsB[0:TAG, 0, :])            # t=128 (gated)

    tr = transitions.astype(f32, copy=False)
    E = np.exp(tr)
    E[STOP, STOP] = 1.0
    bd = np.zeros((128, 128), f32)
    bd[0:TAG, 0:TAG] = E
    bd[BW : BW + TAG, BW : BW + TAG] = E.T

    colc = np.zeros((128, 3), f32)
    colc[0:TAG, 0] = -C0                      # fwd rows, ungated
    colc[STOP, 0] = -MGATE
    colc[BW : BW + TAG, 0] = -(MGATE + C0)    # bwd rows, gated
    colc[BW + STOP, 0] = 0.0
    colc[0:TAG, 1] = -(MGATE + C0)            # DlastF (fwd t=128 gated) bias
    colc[STOP, 1] = 0.0
    colc[0:TAG, 2] = tr[START, :] - C0        # U0 bias: fold E[START,:] in
    colc[STOP, 2] = -MGATE

    s2bh = np.zeros((2, 128), f32)
    s2bh[0, 0:TAG] = 1.0
    s2bh[1, BW : BW + TAG] = 1.0

    # gold one-hots (masked tags match nothing; prev needs no mask)
    iota1 = np.arange(1, TAG + 1, dtype=f32)
    tgm = (tg + 1.0) * m
    yh = (tgm[..., None] == iota1).astype(bf).reshape(128, M32, TAG)
    prev = np.concatenate([np.full((BL, 1), START, f32), tg[:, :-1]], axis=1)
    ypa = np.zeros((BL, S, TAG + 1), f32)
    ypa[:, :, 1:] = prev[..., None] + 1.0 == iota1
    mnext = np.concatenate([m[:, 1:], np.zeros((BL, 1), f32)], axis=1)
    ypa[:, :, 0] = m - mnext                                      # w_last
    ypah = ypa.astype(bf).reshape(128, M32, TAG + 1)
    fnh = f.astype(bf).reshape(128, M32, TAG)

    trp = np.zeros((TAG + 1, TAG), f32)
    trp[0, :] = tr[:, STOP]
    trp[1:, :] = tr

    return {
        "featsF": featsF,
        "featsB": featsB,
        "featsL": featsL,
        "colc": colc,
        "bdh": bd.astype(bf),
        "s2bh": s2bh,
        "yh": np.ascontiguousarray(yh),
        "ypah": np.ascontiguousarray(ypah),
        "fnh": np.ascontiguousarray(fnh),
        "trph": trp,
    }


def _combine(results, mask):
    """Host-side unshard: logs of staged scales + partial sums -> scalar."""
    lengths = mask.astype(np.int64).sum(axis=1)  # (B,)
    fwd = np.float64(0.0)
    gold = np.float64(0.0)
    for core, res in enumerate(results):
        sc = res["out_scan"].astype(np.float64)      # (2, 2*BL)
        gl = res["out_gold"].astype(np.float64)      # (1, 4)
        sf, sb, ssum = sc[0, 0:BL], sc[1, 0:BL], sc[0, BL : 2 * BL]
        lens = lengths[core * BL : (core + 1) * BL].astype(np.float64)
        fwd += (np.log(ssum) + np.log(sf) + np.log(sb) + C0 * lens).sum()
        gold += gl[0, 0] + gl[0, 1]
    return np.asarray(fwd - gold, dtype=np.float32)[()]


def kernel(feats, transitions, mask, tags):
    feats = np.asarray(feats)
    transitions = np.asarray(transitions)
    mask = np.asarray(mask)
    tags = np.asarray(tags)

    if "nc" not in _CACHE:
        _CACHE["nc"] = _build_nc(debug=False)
    nc = _CACHE["nc"]

    from concourse import bass_utils

    in_maps = [
        _prep_core_inputs(feats, transitions, mask, tags, c) for c in range(NCORES)
    ]
    out = bass_utils.run_bass_kernel_spmd(nc, in_maps, core_ids=list(range(NCORES)))
    return _combine(out.results, mask)


# revision 22
# speedup vs baseline: 1.0100x; 1.0100x over previous
"""Trainium2 Bass kernel for CRF negative log-likelihood (nn_CRF).

Strategy:
  - data-parallel over batch: 8 cores x 16 sequences each.
  - forward algorithm in the exp domain, fwd and bwd chains STACKED on the
    partition axis (fwd state at partitions 0:52, bwd at 64:116) so each of
    the 128 serial steps is ONE bf16 matmul against a constant block-diagonal
    (128,128) stationary + ONE vector multiply by the per-step emission slab.
    bf16 halves PE work (fp32 matmuls are 2 HW passes) and the merged chain
    halves the cross-engine ping-pong count vs separate fwd/bwd chains.
  - no mid-scan renorm: C0 cancels the expected per-step growth, so the
    unnormalized state stays within ~e^{+-40} over 128 steps -- far inside
    fp32/bf16 exponent range; the midpoint dot absorbs the residual scale.
  - masking via the absorbing-STOP construction (lengths >= S/2 so only the
    bwd half and t=128 need mask gating); per-step rescale exp(-C0) folded
    into the emission exps; host adds C0*len back.
  - prologue kept off the chain: constants (block-diagonal exp(trans),
    gold one-hots) are host-marshalled bf16, DMA issue is spread across the
    engine queues, and the emission ACTs are chunked so the scan starts as
    soon as the first slots land.
  - gold path score: one-hot pair counts via 32 PSUM-accumulated bf16
    matmuls that Tile schedules into the PE queue's idle slots (the w_last
    column is folded in as an extra one-hot column so end transitions need
    no separate matmuls); gold elementwise work runs on the otherwise-idle
    GPSIMD engine.
  - device emits per-core partials (renorm scales, midpoint dot, gold sums);
    the host does the final log/sum ("all-reduce").
"""

import numpy as np

TAG = 52
START, STOP = TAG - 2, TAG - 1
B, S = 128, 256
NCORES = 8
BL = B // NCORES            # 16 sequences per core
NK = 128                    # stacked chain length (fwd t=0..128, bwd t=255..129)
GK0 = 40                    # gold pair-matmul emission rides with scan steps
C0 = 4.9                    # constant per-step rescale (nats)
MGATE = 64.0                # mask gate constant (exp(-64) ~ 0)
M32 = (S * BL) // 128       # 32 free columns for (128, M32) gold layout
BW = 64                     # partition offset of the bwd block
FC = 32                     # first-chunk slot count for the emission builds

_CACHE: dict = {}


def _build_nc(debug: bool = False):
    import concourse.bass as bass
    import concourse.mybir as mybir
    import concourse.tile as tile
    from concourse import bacc

    f32 = mybir.dt.float32
    bf16 = mybir.dt.bfloat16
    nc = bacc.Bacc("TRN2", target_bir_lowering=False, debug=debug)

    # ---- external inputs (per-core shards, host-marshalled layouts) ----
    featsF = nc.dram_tensor("featsF", (BW, NK, BL), f32, kind="ExternalInput")
    featsB = nc.dram_tensor("featsB", (BW, NK, BL), f32, kind="ExternalInput")
    featsL = nc.dram_tensor("featsL", (TAG, BL), f32, kind="ExternalInput")
    colc = nc.dram_tensor("colc", (128, 3), f32, kind="ExternalInput")
    bdh = nc.dram_tensor("bdh", (128, 128), bf16, kind="ExternalInput")
    yh = nc.dram_tensor("yh", (128, M32, TAG), bf16, kind="ExternalInput")
    ypah = nc.dram_tensor("ypah", (128, M32, TAG + 1), bf16, kind="ExternalInput")
    fnh = nc.dram_tensor("fnh", (128, M32, TAG), bf16, kind="ExternalInput")
    trph = nc.dram_tensor("trph", (TAG + 1, TAG), f32, kind="ExternalInput")

    # ---- external outputs ----
    # out_scan: row 0 = [renorm fwd scales | Ssum], row 1 = [renorm bwd | 0]
    out_scan = nc.dram_tensor("out_scan", (2, 2 * BL), f32, kind="ExternalOutput")
    # out_gold: [0,0] = emit partial, [0,1] = trans+end partial
    out_gold = nc.dram_tensor("out_gold", (1, 4), f32, kind="ExternalOutput")

    AL = mybir.AluOpType
    EXP = mybir.ActivationFunctionType.Exp

    with tile.TileContext(nc) as tc:
        with (
            tc.tile_pool(name="persist", bufs=1) as persist,
            tc.tile_pool(name="ustate", bufs=3) as ustate,
            tc.tile_pool(name="gold", bufs=1) as goldp,
            tc.tile_pool(name="psZ", bufs=2, space="PSUM") as psZ,
            tc.tile_pool(name="psG", bufs=1, space="PSUM") as psG,
            tc.tile_pool(name="psR", bufs=1, space="PSUM") as psR,
        ):
            # ---------- sync queue: chunked feature DMAs ----------
            # fStack: fwd logits at partitions 0:52, bwd (reversed t, mask
            # gate folded in host-side) at 64:116; chunks aligned with the
            # emission ACT chunks so the scan starts as soon as possible
            fStack = persist.tile([128, NK, BL], f32, name="fStack", tag="fStack")
            nc.sync.dma_start(out=fStack[0:BW, 0:FC, :], in_=featsF[:, 0:FC, :])
            nc.sync.dma_start(out=fStack[BW:128, 0:FC, :], in_=featsB[:, 0:FC, :])
            nc.sync.dma_start(out=fStack[0:BW, FC:NK, :], in_=featsF[:, FC:NK, :])
            nc.sync.dma_start(out=fStack[BW:128, FC:NK, :], in_=featsB[:, FC:NK, :])
            # gold one-hots after the scan-gating chunks (the pair matmuls
            # are dep-pinned behind scan step 40+, so these can land late)
            Ysb = goldp.tile([128, M32, TAG], bf16, name="Ysb", tag="Ysb")
            nc.sync.dma_start(out=Ysb, in_=yh[:, :, :])
            YPsb = goldp.tile([128, M32, TAG + 1], bf16, name="YPsb", tag="YPsb")
            nc.sync.dma_start(out=YPsb, in_=ypah[:, :, :])

            # ---------- scalar queue: colc/BDT DMAs then the emission ACTs ---
            colc_sb = persist.tile([128, 3], f32, name="colc_sb", tag="colc_sb")
            nc.scalar.dma_start(out=colc_sb, in_=colc[:, :])
            BDT = persist.tile([128, 128], bf16, name="BDT", tag="BDT")
            nc.scalar.dma_start(out=BDT, in_=bdh[:, :])
            biasgF = colc_sb[0:TAG, 1:2]           # DlastF bias
            biasU0 = colc_sb[0:TAG, 2:3]           # trans[START,:] - C0

            # ---------- vector queue: small consts only ----------
            onesc = persist.tile([TAG, 1], bf16, name="onesc", tag="onesc")
            nc.vector.memset(onesc, 1.0)
            stage2 = persist.tile([2, 2 * BL], f32, name="stage2", tag="stage2")

            # ---------- gpsimd queue: U0 one-hot, Dst dead rows, late DMAs ---
            U0 = persist.tile([128, BL], bf16, name="U0", tag="U0")
            onebl = persist.tile([128, BL], bf16, name="onebl", tag="onebl")
            nc.gpsimd.memset(onebl, 1.0)
            nc.gpsimd.affine_select(
                out=U0,
                in_=onebl,
                compare_op=AL.is_equal,
                fill=0.0,
                base=-(BW + STOP),
                pattern=[[0, BL]],
                channel_multiplier=1,
            )
            # dead rows 52:64 / 116:128 get exp(0+0)=1 from the 64-row ACTs
            # (zeros marshalled into featsF/featsB rows 52:64); finite is all
            # the chain needs there -- BDT's zero blocks null them out
            Dst = persist.tile([128, NK, BL], f32, name="Dst", tag="Dst")
            fL = persist.tile([TAG, BL], f32, name="fL", tag="fL")
            nc.gpsimd.dma_start(out=fL, in_=featsL[:, :])
            FNsb = goldp.tile([128, M32, TAG], bf16, name="FNsb", tag="FNsb")
            nc.gpsimd.dma_start(out=FNsb, in_=fnh[:, :, :])
            trP = goldp.tile([TAG + 1, TAG], f32, name="trP", tag="trP")
            nc.gpsimd.dma_start(out=trP, in_=trph[:, :])

            # ---------- emission exps (mask gates pre-applied host-side) -----
            # U0 fwd rows = X_0 = exp(f_0 + trans[START,:] - C0) (ACT after the
            # gpsimd one-hot write; Tile orders the overlapping writes)
            nc.scalar.activation(
                out=U0[0:TAG, :], in_=fStack[0:TAG, 0, :], func=EXP, bias=biasU0
            )
            nc.scalar.activation(
                out=Dst[0:BW, 0:FC, :],
                in_=fStack[0:BW, 0:FC, :],
                func=EXP,
                bias=colc_sb[0:BW, 0:1],
            )
            nc.scalar.activation(
                out=Dst[BW:128, 0:FC, :],
                in_=fStack[BW:128, 0:FC, :],
                func=EXP,
                bias=colc_sb[BW:128, 0:1],
            )
            nc.scalar.activation(
                out=Dst[0:BW, FC:NK, :],
                in_=fStack[0:BW, FC:NK, :],
                func=EXP,
                bias=colc_sb[0:BW, 0:1],
            )
            nc.scalar.activation(
                out=Dst[BW:128, FC:NK, :],
                in_=fStack[BW:128, FC:NK, :],
                func=EXP,
                bias=colc_sb[BW:128, 0:1],
            )
            # DlastF: gated d(t=128) at partitions 0:52
            DlastF = persist.tile([TAG, BL], f32, name="DlastF", tag="DlastF")
            nc.scalar.activation(out=DlastF, in_=fL, func=EXP, bias=biasgF)

            # ---------- gold emit partial (gpsimd, fully off-path) ----------
            stage_gold = goldp.tile([1, 4], f32, name="stage_gold", tag="stage_gold")
            nc.gpsimd.memset(stage_gold, 0.0)
            scrap = goldp.tile([128, M32, TAG], f32, name="scrap", tag="scrap")
            nc.gpsimd.tensor_mul(scrap, Ysb, FNsb)
            nc.gpsimd.tensor_reduce(
                out=stage_gold[0:1, 0:1],
                in_=scrap,
                axis=mybir.AxisListType.XYZWC,
                op=AL.add,
            )

            # ================= the stacked scan ==============================
            cnt_a = psG.tile([TAG + 1, TAG], f32, name="cnt_a", tag="cnt_a")
            cnt_sb = goldp.tile([TAG + 1, TAG], f32, name="cnt_sb", tag="cnt_sb")

            Zp = psZ.tile([128, BL], f32, name="Z0", tag="Z")
            nc.tensor.matmul(Zp, BDT, U0, start=True, stop=True)

            Ulast = None
            for k in range(1, NK):
                U = ustate.tile([128, BL], bf16, name=f"U{k}", tag="U")
                nc.vector.tensor_tensor(out=U, in0=Dst[:, k, :], in1=Zp, op=AL.mult)
                Z = psZ.tile([128, BL], f32, name=f"Z{k}", tag="Z")
                zinst = nc.tensor.matmul(Z, BDT, U, start=True, stop=True)
                if k == NK - 1:
                    Ulast = U
                if GK0 <= k < GK0 + 2 * M32 and (k - GK0) % 2 == 0:
                    m = (k - GK0) // 2
                    pinst = nc.tensor.matmul(
                        cnt_a,
                        YPsb[:, m, :],
                        Ysb[:, m, :],
                        start=(m == 0),
                        stop=(m == M32 - 1),
                        skip_group_check=True,
                    )
                    # same-engine ordering edge: without it the static
                    # scheduler hoists the whole pair chain to the front of
                    # the PE queue, where its DMA wait stalls the scan
                    tile.add_dep_helper(
                        pinst.ins, zinst.ins, sync=False,
                        reason="pin gold pair MM behind its scan step",
                    )
                if k == GK0 + 2 * M32 + 1:
                    # bounce the finished pair counts to SBUF for gpsimd
                    nc.vector.tensor_copy(cnt_sb, cnt_a)
                Zp = Z

            # ================= tail ==========================================
            # land the bwd half of Z_127 at partitions 0:52 via PE (column
            # slice of the stationary selects the bwd block)
            ps_zb = psR.tile([TAG, BL], f32, name="ps_zb", tag="ps_zb")
            nc.tensor.matmul(ps_zb, BDT[:, BW : BW + TAG], Ulast, start=True, stop=True)
            U128f = persist.tile([TAG, BL], bf16, name="U128f", tag="U128f")
            nc.vector.tensor_tensor(
                out=U128f, in0=DlastF, in1=Zp[0:TAG, :], op=AL.mult
            )
            Pm = persist.tile([TAG, BL], bf16, name="Pm", tag="Pm")
            nc.vector.tensor_tensor(out=Pm, in0=U128f, in1=ps_zb, op=AL.mult)
            ps_sm = psR.tile([1, BL], f32, name="ps_sm", tag="ps_sm")
            nc.tensor.matmul(ps_sm, onesc, Pm, start=True, stop=True)
            # (DMA cannot read PSUM: bounce through SBUF; unused out_scan
            # region is never read by the host)
            nc.vector.tensor_copy(stage2[0:1, BL : 2 * BL], ps_sm)
            nc.sync.dma_start(out=out_scan[:, :], in_=stage2)

            # gold tail: (pair+end counts) . weights — runs mid-scan
            scrapA = goldp.tile([TAG + 1, TAG], f32, name="scrapA", tag="scrapA")
            nc.gpsimd.tensor_mul(scrapA, cnt_sb, trP)
            nc.gpsimd.tensor_reduce(
                out=stage_gold[0:1, 1:2],
                in_=scrapA,
                axis=mybir.AxisListType.XYZWC,
                op=AL.add,
            )
            nc.sync.dma_start(out=out_gold[:, :], in_=stage_gold)

    nc.compile()
    return nc


def _prep_core_inputs(feats, transitions, mask, tags, core):
    """Host marshalling of the core's batch shard (layouts, dtype casts,
    and the tiny constant tables derived from the 52x52 transitions)."""
    import ml_dtypes

    f32 = np.float32
    bf = ml_dtypes.bfloat16
    sl = slice(core * BL, (core + 1) * BL)
    f = np.ascontiguousarray(feats[sl]).astype(f32, copy=False)   # (BL,S,T)
    m = mask[sl].astype(f32)                                      # (BL,S)
    tg = tags[sl].astype(f32)                                     # (BL,S)

    fT = np.ascontiguousarray(f.transpose(2, 1, 0)).copy()        # (T,S,BL)
    fT[STOP] = 0.0
    idxB = np.array([128] + list(range(S - 1, 128, -1)))          # t=128,255..129
    # fwd/bwd logit blocks padded to 64 partitions (rows 52:64 zero) so the
    # exp ACTs can cover the stacked layout's dead rows; mask gate folded
    # into the bwd block: logits + MGATE*m (-MGATE on the STOP row)
    featsF = np.zeros((BW, NK, BL), f32)
    featsF[0:TAG] = fT[:, 0:NK, :]                                # slots t=0..127
    sg = np.full((TAG, 1, 1), MGATE, f32)
    sg[STOP] = -MGATE
    featsB = np.zeros((BW, NK, BL), f32)
    featsB[0:TAG] = fT[:, idxB, :] + sg * m.T[idxB][None, :, :]
    featsL = np.ascontiguousarray(featsB[0:TAG, 0, :])            # t=128 (gated)

    tr = transitions.astype(f32, copy=False)
    E = np.exp(tr)
    E[STOP, STOP] = 1.0
    bd = np.zeros((128, 128), f32)
    bd[0:TAG, 0:TAG] = E
    bd[BW : BW + TAG, BW : BW + TAG] = E.T

    colc = np.zeros((128, 3), f32)
    colc[0:TAG, 0] = -C0                      # fwd rows, ungated
    colc[STOP, 0] = -MGATE
    colc[BW : BW + TAG, 0] = -(MGATE + C0)    # bwd rows, gated
    colc[BW + STOP, 0] = 0.0
    colc[0:TAG, 1] = -(MGATE + C0)            # DlastF (fwd t=128 gated) bias
    colc[STOP, 1] = 0.0
    colc[0:TAG, 2] = tr[START, :] - C0        # U0 bias: fold E[START,:] in
    colc[STOP, 2] = -MGATE

    # gold one-hots (masked tags match nothing; prev needs no mask)
    iota1 = np.arange(1, TAG + 1, dtype=f32)
    tgm = (tg + 1.0) * m
    yh = (tgm[..., None] == iota1).astype(bf).reshape(128, M32, TAG)
    prev = np.concatenate([np.full((BL, 1), START, f32), tg[:, :-1]], axis=1)
    ypa = np.zeros((BL, S, TAG + 1), f32)
    ypa[:, :, 1:] = prev[..., None] + 1.0 == iota1
    mnext = np.concatenate([m[:, 1:], np.zeros((BL, 1), f32)], axis=1)
    ypa[:, :, 0] = m - mnext                                      # w_last
    ypah = ypa.astype(bf).reshape(128, M32, TAG + 1)
    fnh = f.astype(bf).reshape(128, M32, TAG)

    trp = np.zeros((TAG + 1, TAG), f32)
    trp[0, :] = tr[:, STOP]
    trp[1:, :] = tr

    return {
        "featsF": featsF,
        "featsB": featsB,
        "featsL": featsL,
        "colc": colc,
        "bdh": bd.astype(bf),
        "yh": np.ascontiguousarray(yh),
        "ypah": np.ascontiguousarray(ypah),
        "fnh": np.ascontiguousarray(fnh),
        "trph": trp,
    }


def _combine(results, mask):
    """Host-side unshard: logs of staged scales + partial sums -> scalar."""
    lengths = mask.astype(np.int64).sum(axis=1)  # (B,)
    fwd = np.float64(0.0)
    gold = np.float64(0.0)
    for core, res in enumerate(results):
        sc = res["out_scan"].astype(np.float64)      # (2, 2*BL)
        gl = res["out_gold"].astype(np.float64)      # (1, 4)
        ssum = sc[0, BL : 2 * BL]
        lens = lengths[core * BL : (core + 1) * BL].astype(np.float64)
        fwd += (np.log(ssum) + C0 * lens).sum()
        gold += gl[0, 0] + gl[0, 1]
    return np.asarray(fwd - gold, dtype=np.float32)[()]


def kernel(feats, transitions, mask, tags):
    feats = np.asarray(feats)
    transitions = np.asarray(transitions)
    mask = np.asarray(mask)
    tags = np.asarray(tags)

    if "nc" not in _CACHE:
        _CACHE["nc"] = _build_nc(debug=False)
    nc = _CACHE["nc"]

    from concourse import bass_utils

    in_maps = [
        _prep_core_inputs(feats, transitions, mask, tags, c) for c in range(NCORES)
    ]
    out = bass_utils.run_bass_kernel_spmd(nc, in_maps, core_ids=list(range(NCORES)))
    return _combine(out.results, mask)


# revision 27
# speedup vs baseline: 1.0188x; 1.0087x over previous
"""Trainium2 Bass kernel for CRF negative log-likelihood (nn_CRF).

Strategy:
  - data-parallel over batch: 8 cores x 16 sequences each.
  - forward algorithm in the exp domain, fwd and bwd chains STACKED on the
    partition axis (fwd state at partitions 0:52, bwd at 64:116) so each of
    the 128 serial steps is ONE bf16 matmul against a constant block-diagonal
    (128,128) stationary + ONE vector multiply by the per-step emission slab.
    bf16 halves PE work (fp32 matmuls are 2 HW passes) and the merged chain
    halves the cross-engine ping-pong count vs separate fwd/bwd chains.
  - no mid-scan renorm: C0 cancels the expected per-step growth, so the
    unnormalized state stays within ~e^{+-40} over 128 steps -- far inside
    fp32/bf16 exponent range; the midpoint dot absorbs the residual scale.
  - masking via the absorbing-STOP construction (lengths >= S/2 so only the
    bwd half and t=128 need mask gating); per-step rescale exp(-C0) folded
    into the emission exps; host adds C0*len back.
  - prologue kept off the chain: constants (block-diagonal exp(trans),
    gold one-hots) are host-marshalled bf16, DMA issue is spread across the
    engine queues, and the emission ACTs are chunked so the scan starts as
    soon as the first slots land.
  - gold path score: one-hot pair counts via 32 PSUM-accumulated bf16
    matmuls that Tile schedules into the PE queue's idle slots (the w_last
    column is folded in as an extra one-hot column so end transitions need
    no separate matmuls); gold elementwise work runs on the otherwise-idle
    GPSIMD engine.
  - device emits per-core partials (renorm scales, midpoint dot, gold sums);
    the host does the final log/sum ("all-reduce").
"""

import numpy as np

TAG = 52
START, STOP = TAG - 2, TAG - 1
B, S = 128, 256
NCORES = 8
BL = B // NCORES            # 16 sequences per core
NK = 128                    # stacked chain length (fwd t=0..128, bwd t=255..129)
GK0 = 40                    # gold pair-matmul emission rides with scan steps
C0 = 4.9                    # constant per-step rescale (nats)
MGATE = 64.0                # mask gate constant (exp(-64) ~ 0)
M32 = (S * BL) // 128       # 32 free columns for (128, M32) gold layout
BW = 64                     # partition offset of the bwd block
FC = 32                     # first-chunk slot count for the emission builds

_CACHE: dict = {}


def _build_nc(debug: bool = False):
    import concourse.bass as bass
    import concourse.mybir as mybir
    import concourse.tile as tile
    from concourse import bacc

    f32 = mybir.dt.float32
    bf16 = mybir.dt.bfloat16
    nc = bacc.Bacc("TRN2", target_bir_lowering=False, debug=debug)

    # ---- external inputs (per-core shards, host-marshalled layouts) ----
    featsF = nc.dram_tensor("featsF", (BW, NK, BL), f32, kind="ExternalInput")
    featsB = nc.dram_tensor("featsB", (BW, NK, BL), f32, kind="ExternalInput")
    featsL = nc.dram_tensor("featsL", (TAG, BL), f32, kind="ExternalInput")
    colc = nc.dram_tensor("colc", (128, 3), f32, kind="ExternalInput")
    bdh = nc.dram_tensor("bdh", (128, 128), bf16, kind="ExternalInput")
    yh = nc.dram_tensor("yh", (128, M32, TAG), bf16, kind="ExternalInput")
    ypah = nc.dram_tensor("ypah", (128, M32, TAG + 1), bf16, kind="ExternalInput")
    fnh = nc.dram_tensor("fnh", (128, M32, TAG), bf16, kind="ExternalInput")
    trph = nc.dram_tensor("trph", (TAG + 1, TAG), f32, kind="ExternalInput")

    # ---- external outputs ----
    # out_scan: row 0 = [renorm fwd scales | Ssum], row 1 = [renorm bwd | 0]
    out_scan = nc.dram_tensor("out_scan", (2, 2 * BL), f32, kind="ExternalOutput")
    # out_gold: [0,0] = emit partial, [0,1] = trans+end partial
    out_gold = nc.dram_tensor("out_gold", (1, 4), f32, kind="ExternalOutput")

    AL = mybir.AluOpType
    EXP = mybir.ActivationFunctionType.Exp

    with tile.TileContext(nc) as tc:
        with (
            tc.tile_pool(name="persist", bufs=1) as persist,
            tc.tile_pool(name="ustate", bufs=3) as ustate,
            tc.tile_pool(name="gold", bufs=1) as goldp,
            tc.tile_pool(name="psZ", bufs=2, space="PSUM") as psZ,
            tc.tile_pool(name="psG", bufs=1, space="PSUM") as psG,
            tc.tile_pool(name="psR", bufs=1, space="PSUM") as psR,
        ):
            # ---------- sync queue: chunked feature DMAs ----------
            # fStack: fwd logits at partitions 0:52, bwd (reversed t, mask
            # gate folded in host-side) at 64:116; chunks aligned with the
            # emission ACT chunks so the scan starts as soon as possible
            fStack = persist.tile([128, NK, BL], f32, name="fStack", tag="fStack")
            nc.sync.dma_start(out=fStack[0:BW, 0:FC, :], in_=featsF[:, 0:FC, :])
            nc.sync.dma_start(out=fStack[BW:128, 0:FC, :], in_=featsB[:, 0:FC, :])
            nc.sync.dma_start(out=fStack[0:BW, FC:NK, :], in_=featsF[:, FC:NK, :])
            nc.sync.dma_start(out=fStack[BW:128, FC:NK, :], in_=featsB[:, FC:NK, :])
            # gold one-hots after the scan-gating chunks (the pair matmuls
            # are dep-pinned behind scan step 40+, so these can land late)
            Ysb = goldp.tile([128, M32, TAG], bf16, name="Ysb", tag="Ysb")
            nc.sync.dma_start(out=Ysb, in_=yh[:, :, :])
            YPsb = goldp.tile([128, M32, TAG + 1], bf16, name="YPsb", tag="YPsb")
            nc.sync.dma_start(out=YPsb, in_=ypah[:, :, :])

            # ---------- scalar queue: colc/BDT DMAs then the emission ACTs ---
            colc_sb = persist.tile([128, 3], f32, name="colc_sb", tag="colc_sb")
            nc.scalar.dma_start(out=colc_sb, in_=colc[:, :])
            BDT = persist.tile([128, 128], bf16, name="BDT", tag="BDT")
            nc.scalar.dma_start(out=BDT, in_=bdh[:, :])
            biasgF = colc_sb[0:TAG, 1:2]           # DlastF bias
            biasU0 = colc_sb[0:TAG, 2:3]           # trans[START,:] - C0

            # ---------- vector queue: small consts only ----------
            onesc = persist.tile([TAG, 1], bf16, name="onesc", tag="onesc")
            nc.vector.memset(onesc, 1.0)
            stage2 = persist.tile([2, 2 * BL], f32, name="stage2", tag="stage2")

            # ---------- gpsimd queue: U0 one-hot, Dst dead rows, late DMAs ---
            U0 = persist.tile([128, BL], bf16, name="U0", tag="U0")
            onebl = persist.tile([128, BL], bf16, name="onebl", tag="onebl")
            nc.gpsimd.memset(onebl, 1.0)
            nc.gpsimd.affine_select(
                out=U0,
                in_=onebl,
                compare_op=AL.is_equal,
                fill=0.0,
                base=-(BW + STOP),
                pattern=[[0, BL]],
                channel_multiplier=1,
            )
            # dead rows 52:64 / 116:128 get exp(0+0)=1 from the 64-row ACTs
            # (zeros marshalled into featsF/featsB rows 52:64); finite is all
            # the chain needs there -- BDT's zero blocks null them out
            Dst = persist.tile([128, NK, BL], f32, name="Dst", tag="Dst")
            fL = persist.tile([TAG, BL], f32, name="fL", tag="fL")
            nc.gpsimd.dma_start(out=fL, in_=featsL[:, :])
            FNsb = goldp.tile([128, M32, TAG], bf16, name="FNsb", tag="FNsb")
            nc.gpsimd.dma_start(out=FNsb, in_=fnh[:, :, :])
            trP = goldp.tile([TAG + 1, TAG], f32, name="trP", tag="trP")
            nc.gpsimd.dma_start(out=trP, in_=trph[:, :])

            # ---------- emission exps (mask gates pre-applied host-side) -----
            # U0 fwd rows = X_0 = exp(f_0 + trans[START,:] - C0) (ACT after the
            # gpsimd one-hot write; Tile orders the overlapping writes)
            nc.scalar.activation(
                out=U0[0:TAG, :], in_=fStack[0:TAG, 0, :], func=EXP, bias=biasU0
            )
            # chunk-1 exps split 8/24 (same DMAs, smaller first ACTs) so
            # TT_1 is gated by ~0.4us of ACT work instead of ~1.4us
            for k0, k1 in ((0, 8), (8, FC)):
                nc.scalar.activation(
                    out=Dst[0:BW, k0:k1, :],
                    in_=fStack[0:BW, k0:k1, :],
                    func=EXP,
                    bias=colc_sb[0:BW, 0:1],
                )
                nc.scalar.activation(
                    out=Dst[BW:128, k0:k1, :],
                    in_=fStack[BW:128, k0:k1, :],
                    func=EXP,
                    bias=colc_sb[BW:128, 0:1],
                )
            nc.scalar.activation(
                out=Dst[0:BW, FC:NK, :],
                in_=fStack[0:BW, FC:NK, :],
                func=EXP,
                bias=colc_sb[0:BW, 0:1],
            )
            nc.scalar.activation(
                out=Dst[BW:128, FC:NK, :],
                in_=fStack[BW:128, FC:NK, :],
                func=EXP,
                bias=colc_sb[BW:128, 0:1],
            )
            # DlastF: gated d(t=128) at partitions 0:52
            DlastF = persist.tile([TAG, BL], f32, name="DlastF", tag="DlastF")
            nc.scalar.activation(out=DlastF, in_=fL, func=EXP, bias=biasgF)

            # ---------- gold emit partial (gpsimd, fully off-path) ----------
            stage_gold = goldp.tile([1, 4], f32, name="stage_gold", tag="stage_gold")
            nc.gpsimd.memset(stage_gold, 0.0)
            scrap = goldp.tile([128, M32, TAG], f32, name="scrap", tag="scrap")
            nc.gpsimd.tensor_mul(scrap, Ysb, FNsb)
            nc.gpsimd.tensor_reduce(
                out=stage_gold[0:1, 0:1],
                in_=scrap,
                axis=mybir.AxisListType.XYZWC,
                op=AL.add,
            )

            # ================= the stacked scan ==============================
            cnt_a = psG.tile([TAG + 1, TAG], f32, name="cnt_a", tag="cnt_a")
            cnt_sb = goldp.tile([TAG + 1, TAG], f32, name="cnt_sb", tag="cnt_sb")

            Zp = psZ.tile([128, BL], f32, name="Z0", tag="Z")
            nc.tensor.matmul(Zp, BDT, U0, start=True, stop=True)

            Ulast = None
            for k in range(1, NK):
                U = ustate.tile([128, BL], bf16, name=f"U{k}", tag="U")
                nc.vector.tensor_tensor(out=U, in0=Dst[:, k, :], in1=Zp, op=AL.mult)
                Z = psZ.tile([128, BL], f32, name=f"Z{k}", tag="Z")
                zinst = nc.tensor.matmul(Z, BDT, U, start=True, stop=True)
                if k == NK - 1:
                    Ulast = U
                if GK0 <= k < GK0 + 2 * M32 and (k - GK0) % 2 == 0:
                    m = (k - GK0) // 2
                    pinst = nc.tensor.matmul(
                        cnt_a,
                        YPsb[:, m, :],
                        Ysb[:, m, :],
                        start=(m == 0),
                        stop=(m == M32 - 1),
                        skip_group_check=True,
                    )
                    # same-engine ordering edge: without it the static
                    # scheduler hoists the whole pair chain to the front of
                    # the PE queue, where its DMA wait stalls the scan
                    tile.add_dep_helper(
                        pinst.ins, zinst.ins, sync=False,
                        reason="pin gold pair MM behind its scan step",
                    )
                if k == GK0 + 2 * M32 + 1:
                    # bounce the finished pair counts to SBUF for gpsimd
                    nc.vector.tensor_copy(cnt_sb, cnt_a)
                Zp = Z

            # ================= tail ==========================================
            # land the bwd half of Z_127 at partitions 0:52 via PE (column
            # slice of the stationary selects the bwd block)
            ps_zb = psR.tile([TAG, BL], f32, name="ps_zb", tag="ps_zb")
            nc.tensor.matmul(ps_zb, BDT[:, BW : BW + TAG], Ulast, start=True, stop=True)
            U128f = persist.tile([TAG, BL], bf16, name="U128f", tag="U128f")
            nc.vector.tensor_tensor(
                out=U128f, in0=DlastF, in1=Zp[0:TAG, :], op=AL.mult
            )
            Pm = persist.tile([TAG, BL], bf16, name="Pm", tag="Pm")
            nc.vector.tensor_tensor(out=Pm, in0=U128f, in1=ps_zb, op=AL.mult)
            ps_sm = psR.tile([1, BL], f32, name="ps_sm", tag="ps_sm")
            nc.tensor.matmul(ps_sm, onesc, Pm, start=True, stop=True)
            # (DMA cannot read PSUM: bounce through SBUF; unused out_scan
            # region is never read by the host)
            nc.vector.tensor_copy(stage2[0:1, BL : 2 * BL], ps_sm)
            nc.sync.dma_start(out=out_scan[:, :], in_=stage2)

            # gold tail: (pair+end counts) . weights — runs mid-scan
            scrapA = goldp.tile([TAG + 1, TAG], f32, name="scrapA", tag="scrapA")
            nc.gpsimd.tensor_mul(scrapA, cnt_sb, trP)
            nc.gpsimd.tensor_reduce(
                out=stage_gold[0:1, 1:2],
                in_=scrapA,
                axis=mybir.AxisListType.XYZWC,
                op=AL.add,
            )
            nc.sync.dma_start(out=out_gold[:, :], in_=stage_gold)

    nc.compile()
    return nc


def _prep_core_inputs(feats, transitions, mask, tags, core):
    """Host marshalling of the core's batch shard (layouts, dtype casts,
    and the tiny constant tables derived from the 52x52 transitions)."""
    import ml_dtypes

    f32 = np.float32
    bf = ml_dtypes.bfloat16
    sl = slice(core * BL, (core + 1) * BL)
    f = np.ascontiguousarray(feats[sl]).astype(f32, copy=False)   # (BL,S,T)
    m = mask[sl].astype(f32)                                      # (BL,S)
    tg = tags[sl].astype(f32)                                     # (BL,S)

    fT = np.ascontiguousarray(f.transpose(2, 1, 0)).copy()        # (T,S,BL)
    fT[STOP] = 0.0
    idxB = np.array([128] + list(range(S - 1, 128, -1)))          # t=128,255..129
    # fwd/bwd logit blocks padded to 64 partitions (rows 52:64 zero) so the
    # exp ACTs can cover the stacked layout's dead rows; mask gate folded
    # into the bwd block: logits + MGATE*m (-MGATE on the STOP row)
    featsF = np.zeros((BW, NK, BL), f32)
    featsF[0:TAG] = fT[:, 0:NK, :]                                # slots t=0..127
    sg = np.full((TAG, 1, 1), MGATE, f32)
    sg[STOP] = -MGATE
    featsB = np.zeros((BW, NK, BL), f32)
    featsB[0:TAG] = fT[:, idxB, :] + sg * m.T[idxB][None, :, :]
    featsL = np.ascontiguousarray(featsB[0:TAG, 0, :])            # t=128 (gated)

    tr = transitions.astype(f32, copy=False)
    E = np.exp(tr)
    E[STOP, STOP] = 1.0
    bd = np.zeros((128, 128), f32)
    bd[0:TAG, 0:TAG] = E
    bd[BW : BW + TAG, BW : BW + TAG] = E.T

    colc = np.zeros((128, 3), f32)
    colc[0:TAG, 0] = -C0                      # fwd rows, ungated
    colc[STOP, 0] = -MGATE
    colc[BW : BW + TAG, 0] = -(MGATE + C0)    # bwd rows, gated
    colc[BW + STOP, 0] = 0.0
    colc[0:TAG, 1] = -(MGATE + C0)            # DlastF (fwd t=128 gated) bias
    colc[STOP, 1] = 0.0
    colc[0:TAG, 2] = tr[START, :] - C0        # U0 bias: fold E[START,:] in
    colc[STOP, 2] = -MGATE

    # gold one-hots (masked tags match nothing; prev needs no mask)
    iota1 = np.arange(1, TAG + 1, dtype=f32)
    tgm = (tg + 1.0) * m
    yh = (tgm[..., None] == iota1).astype(bf).reshape(128, M32, TAG)
    prev = np.concatenate([np.full((BL, 1), START, f32), tg[:, :-1]], axis=1)
    ypa = np.zeros((BL, S, TAG + 1), f32)
    ypa[:, :, 1:] = prev[..., None] + 1.0 == iota1
    mnext = np.concatenate([m[:, 1:], np.zeros((BL, 1), f32)], axis=1)
    ypa[:, :, 0] = m - mnext                                      # w_last
    ypah = ypa.astype(bf).reshape(128, M32, TAG + 1)
    fnh = f.astype(bf).reshape(128, M32, TAG)

    trp = np.zeros((TAG + 1, TAG), f32)
    trp[0, :] = tr[:, STOP]
    trp[1:, :] = tr

    return {
        "featsF": featsF,
        "featsB": featsB,
        "featsL": featsL,
        "colc": colc,
        "bdh": bd.astype(bf),
        "yh": np.ascontiguousarray(yh),
        "ypah": np.ascontiguousarray(ypah),
        "fnh": np.ascontiguousarray(fnh),
        "trph": trp,
    }


def _combine(results, mask):
    """Host-side unshard: logs of staged scales + partial sums -> scalar."""
    lengths = mask.astype(np.int64).sum(axis=1)  # (B,)
    fwd = np.float64(0.0)
    gold = np.float64(0.0)
    for core, res in enumerate(results):
        sc = res["out_scan"].astype(np.float64)      # (2, 2*BL)
        gl = res["out_gold"].astype(np.float64)      # (1, 4)
        ssum = sc[0, BL : 2 * BL]
        lens = lengths[core * BL : (core + 1) * BL].astype(np.float64)
        fwd += (np.log(ssum) + C0 * lens).sum()
        gold += gl[0, 0] + gl[0, 1]
    return np.asarray(fwd - gold, dtype=np.float32)[()]


def kernel(feats, transitions, mask, tags):
    feats = np.asarray(feats)
    transitions = np.asarray(transitions)
    mask = np.asarray(mask)
    tags = np.asarray(tags)

    if "nc" not in _CACHE:
        _CACHE["nc"] = _build_nc(debug=False)
    nc = _CACHE["nc"]

    from concourse import bass_utils

    in_maps = [
        _prep_core_inputs(feats, transitions, mask, tags, c) for c in range(NCORES)
    ]
    out = bass_utils.run_bass_kernel_spmd(nc, in_maps, core_ids=list(range(NCORES)))
    return _combine(out.results, mask)
